# revision 21
# baseline (speedup 1.0000x reference)
"""Deformable-transformer encoder kernel for TRN2 (8 NeuronCores, batch-parallel).

Each core processes one batch element end-to-end (B=8 == n_cores).
Feature-major layout: X [128 part x 2 k-tiles x 3840 tok] f16.

Deformable sampling: per-query 10-row windows (start = round(ref*T)-5, host
computed) gathered by indirect DMA from a DRAM value buffer (2 levels per
DMA); bilinear interp realized as tent weights relu(1-|w - posl|) with posl
window-local (pb - S baked into the host-precomputed positional projection).
Tent/coefficient math is batched across all 30 query tiles per (level,point)
with pair-duplicated operands so DVE runs in 2x mode; the window combine
multiply runs at 2x via duplicated coefficient pairs; reduction is split
between GPSIMD (level pair sum) and DVE (add tree). Softmax normalization is
deferred (unnormalized exp folded with a reciprocal after the combine).
LayerNorm uses ones-selector matmuls into PSUM rows + selector-matmul
broadcast (no DRAM round trips).
"""

import sys
sys.path.insert(0, "/opt/trn_rl_repo")

import numpy as np
from contextlib import ExitStack

import concourse.bass as bass
import concourse.bacc as bacc
import concourse.tile as tile
from concourse import mybir
from concourse.bass_utils import run_bass_kernel_spmd
from concourse.masks import make_identity

F32 = mybir.dt.float32
F16 = mybir.dt.float16
I32 = mybir.dt.int32
I16 = mybir.dt.int16

B = 8
D = 256
H = 8
NL = 4
NP = 4
LAYERS = 6
FF = 1024
LENS = [2048, 1024, 512, 256]
T = sum(LENS)            # 3840
QT = T // 128            # 30
HLP = H * NL * NP        # 128
LB = [0, 2048, 3072, 3584]
EPS = 1e-5
W = 8                    # window rows per (query, level)
TC = 480                 # token chunk for LN / FFN
NTC = T // TC            # 8

AF = mybir.ActivationFunctionType
ALU = mybir.AluOpType
AX = mybir.AxisListType


def bap(a, dims, off=None):
    """manually-constructed AP view (list of [step, count], partition first)"""
    return bass.AP(tensor=a.tensor, offset=a.offset if off is None else off, ap=dims)


def build_program():
    nc = bacc.Bacc("TRN2", target_bir_lowering=False, num_swdge_queues=4)

    # ---------------- DRAM parameters ----------------
    x0_d = nc.declare_dram_parameter("x0", [2, 128, T], F16, isOutput=False)
    poa_d = nc.declare_dram_parameter("poa", [LAYERS, 128, QT * D], F16, isOutput=False)
    wsi_d = nc.declare_dram_parameter("wsi", [128, QT, 2, 16], I16, isOutput=False)
    sel_d = nc.declare_dram_parameter("sel", [8, NTC, 128], F16, isOutput=False)
    stsel_d = nc.declare_dram_parameter("stsel", [128, NTC, 8], F16, isOutput=False)
    wvt_d = nc.declare_dram_parameter("wvt", [LAYERS, 2, 128, D], F16, isOutput=False)
    woa_d = nc.declare_dram_parameter("woa", [LAYERS, 2, 128, D], F16, isOutput=False)
    wot_d = nc.declare_dram_parameter("wot", [LAYERS, 2, 128, D], F16, isOutput=False)
    w1t_d = nc.declare_dram_parameter("w1t", [LAYERS, 2, 128, FF], F16, isOutput=False)
    w2t_d = nc.declare_dram_parameter("w2t", [LAYERS, FF // 128, 128, D], F16, isOutput=False)
    bo_d = nc.declare_dram_parameter("bo", [LAYERS, 2, 128, 1], F32, isOutput=False)
    bor_d = nc.declare_dram_parameter("bor", [LAYERS, 1, 2, 128], F16, isOutput=False)
    b2r_d = nc.declare_dram_parameter("b2r", [LAYERS, 1, 2, 128], F16, isOutput=False)
    b1_d = nc.declare_dram_parameter("b1", [LAYERS, FF // 128, 128, 1], F32, isOutput=False)
    b2_d = nc.declare_dram_parameter("b2", [LAYERS, 2, 128, 1], F32, isOutput=False)
    g1_d = nc.declare_dram_parameter("g1", [LAYERS, 2, 128, 1], F32, isOutput=False)
    be1_d = nc.declare_dram_parameter("be1", [LAYERS, 2, 128, 1], F32, isOutput=False)
    g2_d = nc.declare_dram_parameter("g2", [LAYERS, 2, 128, 1], F32, isOutput=False)
    be2_d = nc.declare_dram_parameter("be2", [LAYERS, 2, 128, 1], F32, isOutput=False)
    out_d = nc.declare_dram_parameter("out", [2, 128, T], F16, isOutput=True)

    val_ds = [nc.dram_tensor("val_d0", [T, D], F16),
              nc.dram_tensor("val_d1", [T, D], F16)]
    abrow_d = nc.dram_tensor("abrow_d", [NTC, 2, TC], F16)

    ctx = ExitStack()
    # ---------------- persistent SBUF ----------------
    X = ctx.enter_context(nc.sbuf_tensor("X", [128, 2, T], F16))
    XH = ctx.enter_context(nc.sbuf_tensor("XH", [128, 2, T], F16))
    POSLD = ctx.enter_context(nc.sbuf_tensor("POSLD", [128, QT, 128, 2], F16))
    EXPD = ctx.enter_context(nc.sbuf_tensor("EXPD", [128, QT, 128, 2], F16))
    CW = ctx.enter_context(nc.sbuf_tensor("CW", [128, QT, NL, H, W], F16))
    RD = ctx.enter_context(nc.sbuf_tensor("RD", [128, QT, H], F32))
    WSI = ctx.enter_context(nc.sbuf_tensor("WSI", [128, QT, 2, 16], I16))
    WIT = ctx.enter_context(nc.sbuf_tensor("WIT", [128, W], F16))
    ONESR = ctx.enter_context(nc.sbuf_tensor("ONESR", [1, TC], F16))
    SEL = ctx.enter_context(nc.sbuf_tensor("SEL", [8, NTC, 128], F16))
    STSEL = ctx.enter_context(nc.sbuf_tensor("STSEL", [128, NTC, 8], F16))
    IDEN = ctx.enter_context(nc.sbuf_tensor("IDEN", [128, 128], F32))
    IDENH = ctx.enter_context(nc.sbuf_tensor("IDENH", [128, 128], F16))

    with tile.TileContext(nc) as tc, \
            tc.tile_pool(name="wpool", bufs=2) as wpool, \
            tc.tile_pool(name="poap", bufs=1) as poap, \
            tc.tile_pool(name="work", bufs=2) as work, \
            tc.tile_pool(name="tenp", bufs=2) as tenp, \
            tc.tile_pool(name="gpool", bufs=2) as gpool, \
            tc.tile_pool(name="tmpp", bufs=2) as tmpp, \
            tc.tile_pool(name="cwdp", bufs=2) as cwdp, \
            tc.tile_pool(name="redp", bufs=1) as redp, \
            tc.tile_pool(name="hpool", bufs=1) as hpool, \
            tc.tile_pool(name="lnp", bufs=2) as lnp, \
            tc.tile_pool(name="lnrow", bufs=1) as lnrow, \
            tc.tile_pool(name="lnab", bufs=2) as lnab, \
            tc.tile_pool(name="pmm", bufs=2, space="PSUM") as pmm, \
            tc.tile_pool(name="pffn", bufs=2, space="PSUM") as pffn, \
            tc.tile_pool(name="pt", bufs=2, space="PSUM") as pt, \
            tc.tile_pool(name="pstat", bufs=1, space="PSUM") as pstat:

        # ---- init ----
        make_identity(nc, IDEN[:, :])
        nc.vector.tensor_copy(IDENH[:, :], IDEN[:, :])
        nc.sync.dma_start(out=X[:, :, :], in_=x0_d[:, :, :].rearrange("k p t -> p k t"))
        nc.sync.dma_start(out=WSI[:, :, :, :], in_=wsi_d[:, :, :, :])
        for w in range(W):
            nc.vector.memset(WIT[:, w:w + 1], float(w))
        nc.vector.memset(ONESR[:, :], 1.0)
        nc.sync.dma_start(out=SEL[:, :, :], in_=sel_d[:, :, :])
        nc.sync.dma_start(out=STSEL[:, :, :], in_=stsel_d[:, :, :])

        def layer_norm(src, dst, g_ap, be_ap):
            """LN over feature dim (128 partitions x 2 k tiles) of src -> dst (f16)."""
            # stats: per chunk c, selector matmuls accumulate sum / sumsq rows
            # into PSUM [8, TC] (row c <- chunk c)
            ps_s = pstat.tile([NTC, TC], F32, tag="lns")
            ps_q = pstat.tile([NTC, TC], F32, tag="lnq")
            for c in range(NTC):
                s = slice(c * TC, (c + 1) * TC)
                xsq = lnp.tile([128, 2, TC], F16, tag="xsq")
                for k in range(2):
                    nc.scalar.activation(xsq[:, k, :], src[:, k, s], AF.Square)
                for k in range(2):
                    nc.tensor.matmul(ps_s[:, :], lhsT=STSEL[:, c, :], rhs=src[:, k, s],
                                     start=(c == 0 and k == 0),
                                     stop=(c == NTC - 1 and k == 1))
                for k in range(2):
                    nc.tensor.matmul(ps_q[:, :], lhsT=STSEL[:, c, :], rhs=xsq[:, k, :],
                                     start=(c == 0 and k == 0),
                                     stop=(c == NTC - 1 and k == 1))
            # a,b on [8, TC]
            mu_n = lnrow.tile([NTC, TC], F32, tag="mun")
            nc.vector.tensor_scalar_mul(mu_n[:, :], ps_s[:, :], -1.0 / D)
            va = lnrow.tile([NTC, TC], F32, tag="va")
            nc.vector.tensor_scalar_mul(va[:, :], ps_q[:, :], 1.0 / D)
            msq = lnrow.tile([NTC, TC], F32, tag="msq")
            nc.vector.tensor_tensor(out=msq[:, :], in0=mu_n[:, :], in1=mu_n[:, :], op=ALU.mult)
            nc.vector.tensor_tensor(out=va[:, :], in0=va[:, :], in1=msq[:, :], op=ALU.subtract)
            nc.vector.tensor_scalar_add(va[:, :], va[:, :], EPS)
            sd = lnrow.tile([NTC, TC], F32, tag="sd")
            nc.scalar.activation(sd[:, :], va[:, :], AF.Sqrt)
            ab = lnrow.tile([NTC, 2, TC], F16, tag="ab")
            ar = lnrow.tile([NTC, TC], F32, tag="ar")
            nc.vector.reciprocal(ar[:, :], sd[:, :])
            nc.vector.tensor_copy(ab[:, 0, :], ar[:, :])
            br = lnrow.tile([NTC, TC], F32, tag="br")
            nc.vector.tensor_tensor(out=br[:, :], in0=mu_n[:, :], in1=ar[:, :], op=ALU.mult)
            nc.vector.tensor_copy(ab[:, 1, :], br[:, :])
            nc.sync.dma_start(out=abrow_d[:, :, :], in_=ab[:, :, :])
            # normalize per chunk: dst = (src*a + b)*g + be
            for c in range(NTC):
                s = slice(c * TC, (c + 1) * TC)
                abc = lnab.tile([128, 2, TC], F16, tag="abc")
                nc.scalar.dma_start(
                    out=abc[:, :, :],
                    in_=bap(abrow_d[c, :, :], [[0, 128], [TC, 2], [1, TC]],
                            off=c * 2 * TC))
                for k in range(2):
                    t1 = lnab.tile([128, TC], F16, tag="lnt1")
                    nc.vector.tensor_tensor(out=t1[:, :], in0=src[:, k, s],
                                            in1=abc[:, 0, :], op=ALU.mult)
                    nc.vector.tensor_tensor(out=t1[:, :], in0=t1[:, :],
                                            in1=abc[:, 1, :], op=ALU.add)
                    nc.vector.tensor_scalar(out=dst[:, k, s], in0=t1[:, :],
                                            scalar1=g_ap[k], scalar2=be_ap[k],
                                            op0=ALU.mult, op1=ALU.add)

        for l in range(LAYERS):
            val_d = val_ds[l % 2]
            # ---- load layer weights ----
            WVT = wpool.tile([128, 2, D], F16, tag="wvt")
            WOA = wpool.tile([128, 2, D], F16, tag="woa")
            WOT = wpool.tile([128, 2, D], F16, tag="wot")
            W1T = wpool.tile([128, 2, FF], F16, tag="w1t")
            W2T = wpool.tile([128, FF // 128, D], F16, tag="w2t")
            nc.sync.dma_start(out=WVT[:, :, :], in_=wvt_d[l].rearrange("k p d -> p k d"))
            nc.sync.dma_start(out=WOA[:, :, :], in_=woa_d[l].rearrange("k p d -> p k d"))
            nc.sync.dma_start(out=WOT[:, :, :], in_=wot_d[l].rearrange("k p d -> p k d"))
            nc.sync.dma_start(out=W1T[:, :, :], in_=w1t_d[l].rearrange("k p d -> p k d"))
            nc.sync.dma_start(out=W2T[:, :, :], in_=w2t_d[l].rearrange("k p d -> p k d"))
            BOR = wpool.tile([1, 2, 128], F16, tag="bor")
            B2R = wpool.tile([1, 2, 128], F16, tag="b2r")
            nc.scalar.dma_start(out=BOR[:, :, :], in_=bor_d[l])
            nc.scalar.dma_start(out=B2R[:, :, :], in_=b2r_d[l])
            BO = wpool.tile([128, 2, 1], F32, tag="bo")
            B1 = wpool.tile([128, FF // 128, 1], F32, tag="b1")
            B2 = wpool.tile([128, 2, 1], F32, tag="b2")
            G1 = wpool.tile([128, 2, 1], F32, tag="g1")
            BE1 = wpool.tile([128, 2, 1], F32, tag="be1")
            G2 = wpool.tile([128, 2, 1], F32, tag="g2")
            BE2 = wpool.tile([128, 2, 1], F32, tag="be2")
            nc.scalar.dma_start(out=BO[:, :, :], in_=bo_d[l].rearrange("k p o -> p k o"))
            nc.scalar.dma_start(out=B1[:, :, :], in_=b1_d[l].rearrange("k p o -> p k o"))
            nc.scalar.dma_start(out=B2[:, :, :], in_=b2_d[l].rearrange("k p o -> p k o"))
            nc.scalar.dma_start(out=G1[:, :, :], in_=g1_d[l].rearrange("k p o -> p k o"))
            nc.scalar.dma_start(out=BE1[:, :, :], in_=be1_d[l].rearrange("k p o -> p k o"))
            nc.scalar.dma_start(out=G2[:, :, :], in_=g2_d[l].rearrange("k p o -> p k o"))
            nc.scalar.dma_start(out=BE2[:, :, :], in_=be2_d[l].rearrange("k p o -> p k o"))
            POA = poap.tile([128, QT * D], F16, tag="poa")
            nc.scalar.dma_start(out=POA[:, :], in_=poa_d[l])

            # ---- OFF/AW projections -> POSLD (dup-packed) + EXP ----
            for ti in range(QT):
                s = slice(ti * 128, (ti + 1) * 128)
                ps = pmm.tile([128, D], F32, tag="mm")
                for k in range(2):
                    nc.tensor.matmul(ps[:, :], lhsT=X[:, k, s], rhs=WOA[:, k, :],
                                     start=(k == 0), stop=(k == 1))
                # posl (duplicated pairs): POSLD[:, ti, s, d2] = ps_off[s] + poa_off[s]
                po = POA[:, ti * D:(ti + 1) * D]
                nc.vector.tensor_tensor(
                    out=POSLD[:, ti, :, :],
                    in0=bap(ps, [ps.ap[0], [1, 128], [0, 2]]),
                    in1=bap(po, [po.ap[0], [1, 128], [0, 2]]),
                    op=ALU.add)
                # logits -> exp
                lg = work.tile([128, 128], F16, tag="lg")
                nc.vector.tensor_tensor(
                    out=lg[:, :],
                    in0=bap(ps, [ps.ap[0], [1, 128]], off=ps.offset + 128),
                    in1=bap(po, [po.ap[0], [1, 128]], off=po.offset + 128),
                    op=ALU.add)
                nc.scalar.activation(EXPD[:, ti, :, 0], lg[:, :], AF.Exp)

            # ---- denominator reciprocals (deferred softmax normalization) ----
            expd_full = EXPD[:, :, :, :]
            nc.vector.tensor_copy(
                bap(expd_full, [expd_full.ap[0], [2, QT * 128]], off=expd_full.offset + 1),
                bap(expd_full, [expd_full.ap[0], [2, QT * 128]], off=expd_full.offset))
            dt = work.tile([128, QT, H], F32, tag="dt")
            expd_ap = EXPD[:, :, :, :]
            nc.vector.tensor_reduce(
                out=dt[:, :, :],
                in_=bap(expd_ap, [expd_ap.ap[0], [256, QT], [32, H], [2, 16]],
                        off=expd_ap.offset),
                axis=AX.X, op=ALU.add)
            nc.vector.reciprocal(RD[:, :, 0:H].rearrange("p t h -> p (t h)"),
                                 dt[:, :, :].rearrange("p t h -> p (t h)"))

            # ---- tents: CW[q, ti, l, h, w] = sum_p exp * relu(1 - |w - posl|) ----
            posld_ap = POSLD[:, :, :, :]
            exp_ap = EXPD[:, :, :, :]
            cw_full = CW[:, :, :, :, :]
            wit_ap = WIT[:, :]
            for li in range(NL):
                for p in range(NP):
                    so = (li * NP + p) * 2   # slot offset in POSLD dup units
                    ten = tenp.tile([128, QT, H, W], F16, tag="ten")
                    # w - posl  (2x: innermost packed pairs)
                    nc.vector.tensor_tensor(
                        out=bap(ten, [ten.ap[0], [H * W, QT], [W, H], [2, W // 2], [1, 2]],
                                off=ten.offset),
                        in0=bap(wit_ap, [wit_ap.ap[0], [0, QT], [0, H], [2, W // 2], [1, 2]]),
                        in1=bap(posld_ap, [posld_ap.ap[0], [256, QT], [32, H], [0, W // 2], [1, 2]],
                                off=posld_ap.offset + so),
                        op=ALU.subtract)
                    flat = ten[:, :, :, :].rearrange("p t h w -> p (t h w)")
                    nc.scalar.activation(flat, flat, AF.Abs)
                    nc.scalar.activation(flat, flat, AF.Relu, bias=1.0, scale=-1.0)
                    # * exp -> accumulate into CW
                    expv = bap(exp_ap, [exp_ap.ap[0], [256, QT], [32, H], [0, W // 2], [1, 2]],
                               off=exp_ap.offset + (li * NP + p) * 2)
                    cw_ap = bap(cw_full, [cw_full.ap[0], [NL * H * W, QT], [W, H], [2, W // 2], [1, 2]],
                                off=cw_full.offset + li * H * W)
                    tv = bap(ten, [ten.ap[0], [H * W, QT], [W, H], [2, W // 2], [1, 2]],
                             off=ten.offset)
                    if p == 0:
                        nc.vector.tensor_tensor(out=cw_ap, in0=tv, in1=expv, op=ALU.mult)
                    else:
                        nc.vector.tensor_tensor(out=tv, in0=tv, in1=expv, op=ALU.mult)
                        nc.vector.tensor_tensor(out=cw_ap, in0=cw_ap, in1=tv, op=ALU.add)

            # ---- VALUE projection -> fp16 rows [T, 256] in DRAM ----
            for ti in range(QT):
                s = slice(ti * 128, (ti + 1) * 128)
                ps = pmm.tile([128, D], F32, tag="mm")
                for k in range(2):
                    nc.tensor.matmul(ps[:, :], lhsT=X[:, k, s], rhs=WVT[:, k, :],
                                     start=(k == 0), stop=(k == 1))
                vt = work.tile([128, D], F16, tag="vt")
                nc.scalar.activation(vt[:, :], ps[:, :], AF.Copy)
                nc.sync.dma_start(out=val_d[ti * 128:(ti + 1) * 128, :], in_=vt[:, :])

            # ---- gather + combine + out-proj + residual per q tile ----
            cw_full2 = CW[:, :, :, :, :]
            rd_ap = RD[:, :, :]
            for ti in range(QT):
                s = slice(ti * 128, (ti + 1) * 128)
                tmps = []
                for half in range(2):
                    lp = half * 2
                    winb = gpool.tile([128, 2, W * D], F16, tag="win")
                    vflat = val_d[:, :]
                    nc.gpsimd.dma_gather(
                        out_ap=winb[:, :, :],
                        in_ap=bap(vflat, [[D, T - W + 1], [1, W * D]], off=0),
                        idxs_ap=WSI[:, ti, half, :],
                        num_idxs=256, num_idxs_reg=256,
                        elem_size=W * D, elem_step=D,
                        queue_num=(2 * ti + half) % 4)
                    # dup-packed coefficients for this level pair
                    cwd = cwdp.tile([128, 2 * H * W * 2], F16, tag="cwd")
                    cw_src = bap(cw_full2, [cw_full2.ap[0], [H * W, 2], [W, H], [1, W], [0, 2]],
                                 off=cw_full2.offset + ti * NL * H * W + lp * H * W)
                    cwd_dst = bap(cwd, [cwd.ap[0], [H * W * 2, 2], [W * 2, H], [2, W], [1, 2]],
                                  off=cwd.offset)
                    if half == 0:
                        nc.scalar.activation(cwd_dst, cw_src, AF.Copy)
                    else:
                        nc.vector.tensor_copy(cwd_dst, cw_src)
                    # multiply (2x: dup pairs align d-pairs against cw dups)
                    tmp = tmpp.tile([128, 2, H * W * 32], F16, tag="tmp")
                    for j in range(2):
                        nc.vector.tensor_tensor(
                            out=bap(tmp, [tmp.ap[0], [W * 32, H], [32, W], [2, 16], [1, 2]],
                                    off=tmp.offset + j * H * W * 32),
                            in0=bap(winb, [winb.ap[0], [32, H], [D, W], [2, 16], [1, 2]],
                                    off=winb.offset + j * W * D),
                            in1=bap(cwd, [cwd.ap[0], [W * 2, H], [2, W], [0, 16], [1, 2]],
                                    off=cwd.offset + j * H * W * 2),
                            op=ALU.mult)
                    tmps.append(tmp)
                # level sums (in-place on tmp halves):
                # Pool: tmp0[l0] += tmp0[l1]; DVE: tmp1[l2] += tmp1[l3]; DVE combine
                nc.gpsimd.tensor_tensor(out=tmps[0][:, 0, :],
                                        in0=tmps[0][:, 0, :], in1=tmps[0][:, 1, :],
                                        op=ALU.add)
                nc.vector.tensor_tensor(out=tmps[1][:, 0, :],
                                        in0=tmps[1][:, 0, :], in1=tmps[1][:, 1, :],
                                        op=ALU.add)
                nc.vector.tensor_tensor(out=tmps[0][:, 0, :],
                                        in0=tmps[0][:, 0, :], in1=tmps[1][:, 0, :],
                                        op=ALU.add)
                ts4f = tmps[0][:, 0, :]
                # w tree: 8 -> 4 -> 2 -> 1
                r4 = redp.tile([128, H, 4, 32], F16, tag="r4")
                nc.vector.tensor_tensor(
                    out=r4[:, :, :, :].rearrange("p h w d -> p (h w d)"),
                    in0=bap(ts4f, [ts4f.ap[0], [W * 32, H], [32, 4], [1, 32]], off=ts4f.offset),
                    in1=bap(ts4f, [ts4f.ap[0], [W * 32, H], [32, 4], [1, 32]],
                            off=ts4f.offset + 4 * 32),
                    op=ALU.add)
                r4f = r4[:, :, :, :].rearrange("p h w d -> p (h w d)")
                r2 = redp.tile([128, H, 2, 32], F16, tag="r2")
                nc.vector.tensor_tensor(
                    out=r2[:, :, :, :].rearrange("p h w d -> p (h w d)"),
                    in0=bap(r4f, [r4f.ap[0], [4 * 32, H], [32, 2], [1, 32]], off=r4f.offset),
                    in1=bap(r4f, [r4f.ap[0], [4 * 32, H], [32, 2], [1, 32]],
                            off=r4f.offset + 2 * 32),
                    op=ALU.add)
                r2f = r2[:, :, :, :].rearrange("p h w d -> p (h w d)")
                rn = work.tile([128, H, 32], F16, tag="rn")
                nc.vector.tensor_tensor(
                    out=rn[:, :, :].rearrange("p h d -> p (h d)"),
                    in0=bap(r2f, [r2f.ap[0], [2 * 32, H], [1, 32]], off=r2f.offset),
                    in1=bap(r2f, [r2f.ap[0], [2 * 32, H], [1, 32]], off=r2f.offset + 32),
                    op=ALU.add)
                # normalize by softmax denominator
                att = work.tile([128, D], F32, tag="att")
                nc.vector.tensor_tensor(
                    out=att[:, :],
                    in0=rn[:, :, :].rearrange("p h d -> p (h d)"),
                    in1=bap(rd_ap, [rd_ap.ap[0], [1, H], [0, 32]], off=rd_ap.offset + ti * H),
                    op=ALU.mult)
                # out projection + residual (X += Wo @ att + bo)
                atb = work.tile([128, 2, 128], F16, tag="atb")
                for k in range(2):
                    trp = pt.tile([128, 128], F32, tag="pt")
                    nc.tensor.transpose(out=trp[:, :], in_=att[:, k * 128:(k + 1) * 128],
                                        identity=IDEN[:, :])
                    nc.scalar.activation(atb[:, k, :], trp[:, :], AF.Copy)
                for m in range(2):
                    ps = pt.tile([128, 128], F32, tag="pt")
                    for k in range(2):
                        nc.tensor.matmul(ps[:, :], lhsT=WOT[:, k, m * 128:(m + 1) * 128],
                                         rhs=atb[:, k, :], start=(k == 0), stop=False)
                    nc.tensor.matmul(ps[:, :], lhsT=BOR[:, m, :], rhs=ONESR[:, 0:128],
                                     start=False, stop=True)
                    nc.vector.tensor_tensor(out=X[:, m, s], in0=X[:, m, s],
                                            in1=ps[:, :], op=ALU.add)

            # ---- LN1 ----
            layer_norm(X, XH, [G1[:, 0, :], G1[:, 1, :]], [BE1[:, 0, :], BE1[:, 1, :]])

            # ---- FFN + residual into X ----
            for c in range(NTC):
                s = slice(c * TC, (c + 1) * TC)
                h1 = hpool.tile([128, FF // 128, TC], F16, tag="h1")
                for m in range(FF // 128):
                    ps = pffn.tile([128, TC], F32, tag="ffn")
                    for k in range(2):
                        nc.tensor.matmul(ps[:, :], lhsT=W1T[:, k, m * 128:(m + 1) * 128],
                                         rhs=XH[:, k, s], start=(k == 0), stop=(k == 1))
                    nc.scalar.activation(h1[:, m, :], ps[:, :], AF.Relu, bias=B1[:, m, :])
                for m in range(2):
                    ps = pffn.tile([128, TC], F32, tag="ffn")
                    for k in range(FF // 128):
                        nc.tensor.matmul(ps[:, :], lhsT=W2T[:, k, m * 128:(m + 1) * 128],
                                         rhs=h1[:, k, :], start=(k == 0), stop=False)
                    nc.tensor.matmul(ps[:, :], lhsT=B2R[:, m, :], rhs=ONESR[:, :],
                                     start=False, stop=True)
                    nc.vector.tensor_tensor(out=X[:, m, s], in0=XH[:, m, s],
                                            in1=ps[:, :], op=ALU.add)

            layer_norm(X, X, [G2[:, 0, :], G2[:, 1, :]], [BE2[:, 0, :], BE2[:, 1, :]])

        # ---- write result ----
        nc.sync.dma_start(out=out_d[:, :, :].rearrange("k p t -> p k t"), in_=X[:, :, :])

    ctx.close()
    nc.finalize()
    return nc


def _prep_core_inputs(b, srcs, poss, masks, level_embed, W_off, b_off, W_aw, b_aw,
                      W_val, b_val, W_out, b_out, g1, be1, W1, b1, W2, b2, g2, be2):
    f32 = np.float32
    x0 = np.concatenate([s[b] for s in srcs], axis=1).astype(f32)       # [256, T]
    pos = np.concatenate(
        [p[b] + level_embed[i][:, None] for i, p in enumerate(poss)], axis=1
    ).astype(f32)                                                        # [256, T]
    vr = np.stack([m[b].sum() / m.shape[1] for m in masks]).astype(f32)  # [NL]
    ref1 = np.concatenate(
        [(np.arange(Tl, dtype=f32) + 0.5) / Tl for Tl in LENS]
    )                                                                    # [T]
    # raw sampling base position per (q, l): ref*T_l - 0.5
    pbq = ref1[:, None] * vr[None, :] * np.array(LENS, f32)[None, :] - 0.5  # [T, NL]
    Sl = np.clip(np.rint(pbq) - (W // 2), 0, np.array(LENS)[None, :] - W)   # [T, NL]
    rows = (Sl + np.array(LB)[None, :]).astype(np.int64)                    # [T, NL]
    # dma_gather index layout: per (tile, level-pair): 256 idxs, idx i at
    # [i % 16, tile, pair, i // 16]; i < 128 -> level 2*pair, else 2*pair+1
    wsi = np.zeros((16, QT, 2, 16), np.int16)
    for ti in range(QT):
        for pair in range(2):
            idx = np.concatenate([rows[ti * 128:(ti + 1) * 128, 2 * pair],
                                  rows[ti * 128:(ti + 1) * 128, 2 * pair + 1]])
            wsi[np.arange(256) % 16, ti, pair, np.arange(256) // 16] = idx.astype(np.int16)
    # ucode reads index stripes per gpsimd core: replicate the 16-partition
    # block across all 128 partitions
    wsi = np.tile(wsi, (8, 1, 1, 1))

    def ktile(w):  # [din, dout] -> [2, 128, dout]
        return np.ascontiguousarray(w.reshape(2, 128, -1))

    wvt = np.stack([ktile(W_val[l].T) for l in range(LAYERS)])
    woa_w = [np.concatenate([W_off[l], W_aw[l]], axis=0) for l in range(LAYERS)]
    woa = np.stack([ktile(w.T) for w in woa_w])
    # poa[l, q, out]: off half gets pb - S baked in (window-local positions)
    pbl = (pbq - Sl).astype(f32)                                         # [T, NL]
    poa = np.stack([
        (woa_w[l] @ pos).T + np.concatenate([b_off[l], b_aw[l]])[None, :]
        for l in range(LAYERS)
    ])                                                                    # [L, T, 256]
    # slot s = h*16 + l*4 + p -> level index (s//4) % 4
    lidx = (np.arange(HLP) // NP) % NL
    poa[:, :, :HLP] += pbl[None, :, lidx]
    poa = poa.astype(np.float16).reshape(LAYERS, QT, 128, 2 * HLP)
    # kernel reads poa as [128 part(q within tile), QT*256]
    poa = np.ascontiguousarray(poa.transpose(0, 2, 1, 3).reshape(LAYERS, 128, QT * 2 * HLP))

    sel = np.zeros((8, NTC, 128), np.float16)
    for c in range(NTC):
        sel[c, c, :] = 1.0
    stsel = np.zeros((128, NTC, 8), np.float16)
    for c in range(NTC):
        stsel[:, c, c] = 1.0

    wot = np.stack([ktile(W_out[l].T) for l in range(LAYERS)])
    bo = b_out.reshape(LAYERS, 2, 128, 1).astype(f32)
    w1t = np.stack([ktile(W1[l].T) for l in range(LAYERS)]).astype(np.float16)
    b1r = b1.reshape(LAYERS, FF // 128, 128, 1).astype(f32)
    w2t = np.stack([np.ascontiguousarray(W2[l].T.reshape(FF // 128, 128, D))
                    for l in range(LAYERS)]).astype(np.float16)
    b2r = b2.reshape(LAYERS, 2, 128, 1).astype(f32)
    return {
        "x0": x0.reshape(2, 128, T).astype(np.float16),
        "poa": poa, "wsi": wsi, "sel": sel, "stsel": stsel,
        "wvt": wvt.astype(np.float16), "woa": woa.astype(np.float16),
        "wot": wot.astype(np.float16), "bo": bo,
        "bor": b_out.reshape(LAYERS, 1, 2, 128).astype(np.float16),
        "b2r": b2.reshape(LAYERS, 1, 2, 128).astype(np.float16),
        "w1t": w1t, "b1": b1r,
        "w2t": w2t, "b2": b2r,
        "g1": g1.reshape(LAYERS, 2, 128, 1).astype(f32),
        "be1": be1.reshape(LAYERS, 2, 128, 1).astype(f32),
        "g2": g2.reshape(LAYERS, 2, 128, 1).astype(f32),
        "be2": be2.reshape(LAYERS, 2, 128, 1).astype(f32),
    }


_NC_CACHE = {}


def _collect_args(inputs):
    return dict(
        srcs=[inputs[f"src{i}"] for i in range(4)],
        poss=[inputs[f"pos{i}"] for i in range(4)],
        masks=[inputs[f"mask{i}"] for i in range(4)],
        level_embed=inputs["level_embed"],
        W_off=inputs["W_off"], b_off=inputs["b_off"],
        W_aw=inputs["W_aw"], b_aw=inputs["b_aw"],
        W_val=inputs["W_val"], b_val=inputs["b_val"],
        W_out=inputs["W_out"], b_out=inputs["b_out"],
        g1=inputs["g1"], be1=inputs["be1"],
        W1=inputs["W1"], b1=inputs["b1"],
        W2=inputs["W2"], b2=inputs["b2"],
        g2=inputs["g2"], be2=inputs["be2"],
    )


def kernel(**inputs):
    inputs = {k: np.asarray(v) for k, v in inputs.items()}
    args = _collect_args(inputs)
    if "nc" not in _NC_CACHE:
        _NC_CACHE["nc"] = build_program()
    nc = _NC_CACHE["nc"]
    in_maps = [_prep_core_inputs(b, **args) for b in range(B)]
    for attempt in range(2):
        res = run_bass_kernel_spmd(nc, in_maps, core_ids=list(range(B)))
        outs = []
        for b in range(B):
            o = res.results[b]["out"]          # [2, 128, T] f16
            outs.append(o.reshape(D, T).T.astype(np.float32))
        out = np.stack(outs)
        if np.isfinite(out).all():
            return out
    return out


if __name__ == "__main__":
    np.random.seed(0)
    build_program()
    print("program built OK")


# revision 23
# speedup vs baseline: 1.2259x; 1.2259x over previous
"""Deformable-transformer encoder kernel for TRN2 (8 NeuronCores, batch-parallel).

Each core processes one batch element end-to-end (B=8 == n_cores).
Feature-major layout: X [128 part x 2 k-tiles x 3840 tok] f16.

Deformable sampling: per-query 10-row windows (start = round(ref*T)-5, host
computed) gathered by indirect DMA from a DRAM value buffer (2 levels per
DMA); bilinear interp realized as tent weights relu(1-|w - posl|) with posl
window-local (pb - S baked into the host-precomputed positional projection).
Tent/coefficient math is batched across all 30 query tiles per (level,point)
with pair-duplicated operands so DVE runs in 2x mode; the window combine
multiply runs at 2x via duplicated coefficient pairs; reduction is split
between GPSIMD (level pair sum) and DVE (add tree). Softmax normalization is
deferred (unnormalized exp folded with a reciprocal after the combine).
LayerNorm uses ones-selector matmuls into PSUM rows + selector-matmul
broadcast (no DRAM round trips).
"""

import sys
sys.path.insert(0, "/opt/trn_rl_repo")

import numpy as np
from contextlib import ExitStack

import concourse.bass as bass
import concourse.bacc as bacc
import concourse.tile as tile
from concourse import mybir
from concourse.bass_utils import run_bass_kernel_spmd
from concourse.masks import make_identity

F32 = mybir.dt.float32
F16 = mybir.dt.float16
I32 = mybir.dt.int32
I16 = mybir.dt.int16

B = 8
D = 256
H = 8
NL = 4
NP = 4
LAYERS = 6
FF = 1024
LENS = [2048, 1024, 512, 256]
T = sum(LENS)            # 3840
QT = T // 128            # 30
HLP = H * NL * NP        # 128
LB = [0, 2048, 3072, 3584]
EPS = 1e-5
W = 8                    # window rows per (query, level)
TC = 480                 # token chunk for LN / FFN
NTC = T // TC            # 8

AF = mybir.ActivationFunctionType
ALU = mybir.AluOpType
AX = mybir.AxisListType


def bap(a, dims, off=None):
    """manually-constructed AP view (list of [step, count], partition first)"""
    return bass.AP(tensor=a.tensor, offset=a.offset if off is None else off, ap=dims)


def build_program():
    nc = bacc.Bacc("TRN2", target_bir_lowering=False, num_swdge_queues=4)

    # ---------------- DRAM parameters ----------------
    x0_d = nc.declare_dram_parameter("x0", [2, 128, T], F16, isOutput=False)
    poa_d = nc.declare_dram_parameter("poa", [LAYERS, 128, QT * D], F16, isOutput=False)
    wsi_d = nc.declare_dram_parameter("wsi", [128, QT, NL], I32, isOutput=False)
    sel_d = nc.declare_dram_parameter("sel", [8, NTC, 128], F16, isOutput=False)
    stsel_d = nc.declare_dram_parameter("stsel", [128, NTC, 8], F16, isOutput=False)
    wvt_d = nc.declare_dram_parameter("wvt", [LAYERS, 2, 128, D], F16, isOutput=False)
    woa_d = nc.declare_dram_parameter("woa", [LAYERS, 2, 128, D], F16, isOutput=False)
    wot_d = nc.declare_dram_parameter("wot", [LAYERS, 2, 128, D], F16, isOutput=False)
    w1t_d = nc.declare_dram_parameter("w1t", [LAYERS, 2, 128, FF], F16, isOutput=False)
    w2t_d = nc.declare_dram_parameter("w2t", [LAYERS, FF // 128, 128, D], F16, isOutput=False)
    bo_d = nc.declare_dram_parameter("bo", [LAYERS, 2, 128, 1], F32, isOutput=False)
    bor_d = nc.declare_dram_parameter("bor", [LAYERS, 1, 2, 128], F16, isOutput=False)
    b2r_d = nc.declare_dram_parameter("b2r", [LAYERS, 1, 2, 128], F16, isOutput=False)
    b1_d = nc.declare_dram_parameter("b1", [LAYERS, FF // 128, 128, 1], F32, isOutput=False)
    b2_d = nc.declare_dram_parameter("b2", [LAYERS, 2, 128, 1], F32, isOutput=False)
    g1_d = nc.declare_dram_parameter("g1", [LAYERS, 2, 128, 1], F32, isOutput=False)
    be1_d = nc.declare_dram_parameter("be1", [LAYERS, 2, 128, 1], F32, isOutput=False)
    g2_d = nc.declare_dram_parameter("g2", [LAYERS, 2, 128, 1], F32, isOutput=False)
    be2_d = nc.declare_dram_parameter("be2", [LAYERS, 2, 128, 1], F32, isOutput=False)
    out_d = nc.declare_dram_parameter("out", [2, 128, T], F16, isOutput=True)

    val_ds = [nc.dram_tensor("val_d0", [T, D], F16),
              nc.dram_tensor("val_d1", [T, D], F16)]
    abrow_d = nc.dram_tensor("abrow_d", [NTC, 2, TC], F16)

    ctx = ExitStack()
    # ---------------- persistent SBUF ----------------
    X = ctx.enter_context(nc.sbuf_tensor("X", [128, 2, T], F16))
    XH = ctx.enter_context(nc.sbuf_tensor("XH", [128, 2, T], F16))
    POSLD = ctx.enter_context(nc.sbuf_tensor("POSLD", [128, QT, 128, 2], F16))
    EXPD = ctx.enter_context(nc.sbuf_tensor("EXPD", [128, QT, 128, 2], F16))
    CW = ctx.enter_context(nc.sbuf_tensor("CW", [128, QT, NL, H, W], F16))
    RD = ctx.enter_context(nc.sbuf_tensor("RD", [128, QT, H], F32))
    WSI = ctx.enter_context(nc.sbuf_tensor("WSI", [128, QT, NL], I32))
    WIT = ctx.enter_context(nc.sbuf_tensor("WIT", [128, W], F16))
    ONESR = ctx.enter_context(nc.sbuf_tensor("ONESR", [1, TC], F16))
    SEL = ctx.enter_context(nc.sbuf_tensor("SEL", [8, NTC, 128], F16))
    STSEL = ctx.enter_context(nc.sbuf_tensor("STSEL", [128, NTC, 8], F16))
    IDEN = ctx.enter_context(nc.sbuf_tensor("IDEN", [128, 128], F32))
    IDENH = ctx.enter_context(nc.sbuf_tensor("IDENH", [128, 128], F16))

    with tile.TileContext(nc) as tc, \
            tc.tile_pool(name="wpool", bufs=2) as wpool, \
            tc.tile_pool(name="poap", bufs=1) as poap, \
            tc.tile_pool(name="work", bufs=2) as work, \
            tc.tile_pool(name="tenp", bufs=2) as tenp, \
            tc.tile_pool(name="gpool", bufs=2) as gpool, \
            tc.tile_pool(name="tmpp", bufs=2) as tmpp, \
            tc.tile_pool(name="cwdp", bufs=2) as cwdp, \
            tc.tile_pool(name="redp", bufs=1) as redp, \
            tc.tile_pool(name="hpool", bufs=1) as hpool, \
            tc.tile_pool(name="lnp", bufs=2) as lnp, \
            tc.tile_pool(name="lnrow", bufs=1) as lnrow, \
            tc.tile_pool(name="lnab", bufs=2) as lnab, \
            tc.tile_pool(name="pmm", bufs=2, space="PSUM") as pmm, \
            tc.tile_pool(name="pffn", bufs=2, space="PSUM") as pffn, \
            tc.tile_pool(name="pt", bufs=2, space="PSUM") as pt, \
            tc.tile_pool(name="pstat", bufs=1, space="PSUM") as pstat:

        # ---- init ----
        make_identity(nc, IDEN[:, :])
        nc.vector.tensor_copy(IDENH[:, :], IDEN[:, :])
        nc.sync.dma_start(out=X[:, :, :], in_=x0_d[:, :, :].rearrange("k p t -> p k t"))
        nc.sync.dma_start(out=WSI[:, :, :], in_=wsi_d[:, :, :])
        for w in range(W):
            nc.vector.memset(WIT[:, w:w + 1], float(w))
        nc.vector.memset(ONESR[:, :], 1.0)
        nc.sync.dma_start(out=SEL[:, :, :], in_=sel_d[:, :, :])
        nc.sync.dma_start(out=STSEL[:, :, :], in_=stsel_d[:, :, :])

        def layer_norm(src, dst, g_ap, be_ap):
            """LN over feature dim (128 partitions x 2 k tiles) of src -> dst (f16)."""
            # stats: per chunk c, selector matmuls accumulate sum / sumsq rows
            # into PSUM [8, TC] (row c <- chunk c)
            ps_s = pstat.tile([NTC, TC], F32, tag="lns")
            ps_q = pstat.tile([NTC, TC], F32, tag="lnq")
            for c in range(NTC):
                s = slice(c * TC, (c + 1) * TC)
                xsq = lnp.tile([128, 2, TC], F16, tag="xsq")
                for k in range(2):
                    nc.scalar.activation(xsq[:, k, :], src[:, k, s], AF.Square)
                for k in range(2):
                    nc.tensor.matmul(ps_s[:, :], lhsT=STSEL[:, c, :], rhs=src[:, k, s],
                                     start=(c == 0 and k == 0),
                                     stop=(c == NTC - 1 and k == 1))
                for k in range(2):
                    nc.tensor.matmul(ps_q[:, :], lhsT=STSEL[:, c, :], rhs=xsq[:, k, :],
                                     start=(c == 0 and k == 0),
                                     stop=(c == NTC - 1 and k == 1))
            # a,b on [8, TC]
            mu_n = lnrow.tile([NTC, TC], F32, tag="mun")
            nc.vector.tensor_scalar_mul(mu_n[:, :], ps_s[:, :], -1.0 / D)
            va = lnrow.tile([NTC, TC], F32, tag="va")
            nc.vector.tensor_scalar_mul(va[:, :], ps_q[:, :], 1.0 / D)
            msq = lnrow.tile([NTC, TC], F32, tag="msq")
            nc.vector.tensor_tensor(out=msq[:, :], in0=mu_n[:, :], in1=mu_n[:, :], op=ALU.mult)
            nc.vector.tensor_tensor(out=va[:, :], in0=va[:, :], in1=msq[:, :], op=ALU.subtract)
            nc.vector.tensor_scalar_add(va[:, :], va[:, :], EPS)
            sd = lnrow.tile([NTC, TC], F32, tag="sd")
            nc.scalar.activation(sd[:, :], va[:, :], AF.Sqrt)
            ab = lnrow.tile([NTC, 2, TC], F16, tag="ab")
            ar = lnrow.tile([NTC, TC], F32, tag="ar")
            nc.vector.reciprocal(ar[:, :], sd[:, :])
            nc.vector.tensor_copy(ab[:, 0, :], ar[:, :])
            br = lnrow.tile([NTC, TC], F32, tag="br")
            nc.vector.tensor_tensor(out=br[:, :], in0=mu_n[:, :], in1=ar[:, :], op=ALU.mult)
            nc.vector.tensor_copy(ab[:, 1, :], br[:, :])
            nc.sync.dma_start(out=abrow_d[:, :, :], in_=ab[:, :, :])
            # normalize per chunk: dst = (src*a + b)*g + be
            for c in range(NTC):
                s = slice(c * TC, (c + 1) * TC)
                abc = lnab.tile([128, 2, TC], F16, tag="abc")
                nc.scalar.dma_start(
                    out=abc[:, :, :],
                    in_=bap(abrow_d[c, :, :], [[0, 128], [TC, 2], [1, TC]],
                            off=c * 2 * TC))
                for k in range(2):
                    t1 = lnab.tile([128, TC], F16, tag="lnt1")
                    nc.vector.tensor_tensor(out=t1[:, :], in0=src[:, k, s],
                                            in1=abc[:, 0, :], op=ALU.mult)
                    nc.vector.tensor_tensor(out=t1[:, :], in0=t1[:, :],
                                            in1=abc[:, 1, :], op=ALU.add)
                    nc.vector.tensor_scalar(out=dst[:, k, s], in0=t1[:, :],
                                            scalar1=g_ap[k], scalar2=be_ap[k],
                                            op0=ALU.mult, op1=ALU.add)

        for l in range(LAYERS):
            val_d = val_ds[l % 2]
            # ---- load layer weights ----
            WVT = wpool.tile([128, 2, D], F16, tag="wvt")
            WOA = wpool.tile([128, 2, D], F16, tag="woa")
            WOT = wpool.tile([128, 2, D], F16, tag="wot")
            W1T = wpool.tile([128, 2, FF], F16, tag="w1t")
            W2T = wpool.tile([128, FF // 128, D], F16, tag="w2t")
            nc.sync.dma_start(out=WVT[:, :, :], in_=wvt_d[l].rearrange("k p d -> p k d"))
            nc.sync.dma_start(out=WOA[:, :, :], in_=woa_d[l].rearrange("k p d -> p k d"))
            nc.sync.dma_start(out=WOT[:, :, :], in_=wot_d[l].rearrange("k p d -> p k d"))
            nc.sync.dma_start(out=W1T[:, :, :], in_=w1t_d[l].rearrange("k p d -> p k d"))
            nc.sync.dma_start(out=W2T[:, :, :], in_=w2t_d[l].rearrange("k p d -> p k d"))
            BOR = wpool.tile([1, 2, 128], F16, tag="bor")
            B2R = wpool.tile([1, 2, 128], F16, tag="b2r")
            nc.scalar.dma_start(out=BOR[:, :, :], in_=bor_d[l])
            nc.scalar.dma_start(out=B2R[:, :, :], in_=b2r_d[l])
            BO = wpool.tile([128, 2, 1], F32, tag="bo")
            B1 = wpool.tile([128, FF // 128, 1], F32, tag="b1")
            B2 = wpool.tile([128, 2, 1], F32, tag="b2")
            G1 = wpool.tile([128, 2, 1], F32, tag="g1")
            BE1 = wpool.tile([128, 2, 1], F32, tag="be1")
            G2 = wpool.tile([128, 2, 1], F32, tag="g2")
            BE2 = wpool.tile([128, 2, 1], F32, tag="be2")
            nc.scalar.dma_start(out=BO[:, :, :], in_=bo_d[l].rearrange("k p o -> p k o"))
            nc.scalar.dma_start(out=B1[:, :, :], in_=b1_d[l].rearrange("k p o -> p k o"))
            nc.scalar.dma_start(out=B2[:, :, :], in_=b2_d[l].rearrange("k p o -> p k o"))
            nc.scalar.dma_start(out=G1[:, :, :], in_=g1_d[l].rearrange("k p o -> p k o"))
            nc.scalar.dma_start(out=BE1[:, :, :], in_=be1_d[l].rearrange("k p o -> p k o"))
            nc.scalar.dma_start(out=G2[:, :, :], in_=g2_d[l].rearrange("k p o -> p k o"))
            nc.scalar.dma_start(out=BE2[:, :, :], in_=be2_d[l].rearrange("k p o -> p k o"))
            POA = poap.tile([128, QT * D], F16, tag="poa")
            nc.scalar.dma_start(out=POA[:, :], in_=poa_d[l])

            # ---- OFF/AW projections -> POSLD (dup-packed) + EXP ----
            for ti in range(QT):
                s = slice(ti * 128, (ti + 1) * 128)
                ps = pmm.tile([128, D], F32, tag="mm")
                for k in range(2):
                    nc.tensor.matmul(ps[:, :], lhsT=X[:, k, s], rhs=WOA[:, k, :],
                                     start=(k == 0), stop=(k == 1))
                # posl (duplicated pairs): POSLD[:, ti, s, d2] = ps_off[s] + poa_off[s]
                po = POA[:, ti * D:(ti + 1) * D]
                nc.vector.tensor_tensor(
                    out=POSLD[:, ti, :, :],
                    in0=bap(ps, [ps.ap[0], [1, 128], [0, 2]]),
                    in1=bap(po, [po.ap[0], [1, 128], [0, 2]]),
                    op=ALU.add)
                # logits -> exp
                lg = work.tile([128, 128], F16, tag="lg")
                nc.vector.tensor_tensor(
                    out=lg[:, :],
                    in0=bap(ps, [ps.ap[0], [1, 128]], off=ps.offset + 128),
                    in1=bap(po, [po.ap[0], [1, 128]], off=po.offset + 128),
                    op=ALU.add)
                nc.scalar.activation(EXPD[:, ti, :, 0], lg[:, :], AF.Exp)

            # ---- denominator reciprocals (deferred softmax normalization) ----
            expd_full = EXPD[:, :, :, :]
            nc.vector.tensor_copy(
                bap(expd_full, [expd_full.ap[0], [2, QT * 128]], off=expd_full.offset + 1),
                bap(expd_full, [expd_full.ap[0], [2, QT * 128]], off=expd_full.offset))
            dt = work.tile([128, QT, H], F32, tag="dt")
            expd_ap = EXPD[:, :, :, :]
            nc.vector.tensor_reduce(
                out=dt[:, :, :],
                in_=bap(expd_ap, [expd_ap.ap[0], [256, QT], [32, H], [2, 16]],
                        off=expd_ap.offset),
                axis=AX.X, op=ALU.add)
            nc.vector.reciprocal(RD[:, :, 0:H].rearrange("p t h -> p (t h)"),
                                 dt[:, :, :].rearrange("p t h -> p (t h)"))

            # ---- tents: CW[q, ti, l, h, w] = sum_p exp * relu(1 - |w - posl|) ----
            posld_ap = POSLD[:, :, :, :]
            exp_ap = EXPD[:, :, :, :]
            cw_full = CW[:, :, :, :, :]
            wit_ap = WIT[:, :]
            for li in range(NL):
                for p in range(NP):
                    so = (li * NP + p) * 2   # slot offset in POSLD dup units
                    ten = tenp.tile([128, QT, H, W], F16, tag="ten")
                    # w - posl  (2x: innermost packed pairs)
                    nc.vector.tensor_tensor(
                        out=bap(ten, [ten.ap[0], [H * W, QT], [W, H], [2, W // 2], [1, 2]],
                                off=ten.offset),
                        in0=bap(wit_ap, [wit_ap.ap[0], [0, QT], [0, H], [2, W // 2], [1, 2]]),
                        in1=bap(posld_ap, [posld_ap.ap[0], [256, QT], [32, H], [0, W // 2], [1, 2]],
                                off=posld_ap.offset + so),
                        op=ALU.subtract)
                    flat = ten[:, :, :, :].rearrange("p t h w -> p (t h w)")
                    nc.scalar.activation(flat, flat, AF.Abs)
                    nc.scalar.activation(flat, flat, AF.Relu, bias=1.0, scale=-1.0)
                    # * exp -> accumulate into CW
                    expv = bap(exp_ap, [exp_ap.ap[0], [256, QT], [32, H], [0, W // 2], [1, 2]],
                               off=exp_ap.offset + (li * NP + p) * 2)
                    cw_ap = bap(cw_full, [cw_full.ap[0], [NL * H * W, QT], [W, H], [2, W // 2], [1, 2]],
                                off=cw_full.offset + li * H * W)
                    tv = bap(ten, [ten.ap[0], [H * W, QT], [W, H], [2, W // 2], [1, 2]],
                             off=ten.offset)
                    if p == 0:
                        nc.vector.tensor_tensor(out=cw_ap, in0=tv, in1=expv, op=ALU.mult)
                    else:
                        nc.vector.tensor_tensor(out=tv, in0=tv, in1=expv, op=ALU.mult)
                        nc.vector.tensor_tensor(out=cw_ap, in0=cw_ap, in1=tv, op=ALU.add)

            # ---- VALUE projection -> fp16 rows [T, 256] in DRAM ----
            for ti in range(QT):
                s = slice(ti * 128, (ti + 1) * 128)
                ps = pmm.tile([128, D], F32, tag="mm")
                for k in range(2):
                    nc.tensor.matmul(ps[:, :], lhsT=X[:, k, s], rhs=WVT[:, k, :],
                                     start=(k == 0), stop=(k == 1))
                vt = work.tile([128, D], F16, tag="vt")
                nc.scalar.activation(vt[:, :], ps[:, :], AF.Copy)
                nc.sync.dma_start(out=val_d[ti * 128:(ti + 1) * 128, :], in_=vt[:, :])

            # ---- gather + combine + out-proj + residual per q tile ----
            cw_full2 = CW[:, :, :, :, :]
            rd_ap = RD[:, :, :]
            for ti in range(QT):
                s = slice(ti * 128, (ti + 1) * 128)
                tmps = []
                for half in range(2):
                    lp = half * 2
                    winb = gpool.tile([128, 2, W * D], F16, tag="win")
                    for j in range(2):
                        nc.gpsimd.indirect_dma_start(
                            out=winb[:, j, :], out_offset=None, in_=val_d[:, :],
                            in_offset=bass.IndirectOffsetOnAxis(
                                ap=WSI[:, ti, lp + j:lp + j + 1], axis=0))
                    # dup-packed coefficients for this level pair
                    cwd = cwdp.tile([128, 2 * H * W * 2], F16, tag="cwd")
                    cw_src = bap(cw_full2, [cw_full2.ap[0], [H * W, 2], [W, H], [1, W], [0, 2]],
                                 off=cw_full2.offset + ti * NL * H * W + lp * H * W)
                    cwd_dst = bap(cwd, [cwd.ap[0], [H * W * 2, 2], [W * 2, H], [2, W], [1, 2]],
                                  off=cwd.offset)
                    if half == 0:
                        nc.scalar.activation(cwd_dst, cw_src, AF.Copy)
                    else:
                        nc.vector.tensor_copy(cwd_dst, cw_src)
                    # multiply (2x: dup pairs align d-pairs against cw dups)
                    tmp = tmpp.tile([128, 2, H * W * 32], F16, tag="tmp")
                    for j in range(2):
                        nc.vector.tensor_tensor(
                            out=bap(tmp, [tmp.ap[0], [W * 32, H], [32, W], [2, 16], [1, 2]],
                                    off=tmp.offset + j * H * W * 32),
                            in0=bap(winb, [winb.ap[0], [32, H], [D, W], [2, 16], [1, 2]],
                                    off=winb.offset + j * W * D),
                            in1=bap(cwd, [cwd.ap[0], [W * 2, H], [2, W], [0, 16], [1, 2]],
                                    off=cwd.offset + j * H * W * 2),
                            op=ALU.mult)
                    tmps.append(tmp)
                # level sums (in-place on tmp halves):
                # Pool: tmp0[l0] += tmp0[l1]; DVE: tmp1[l2] += tmp1[l3]; DVE combine
                nc.gpsimd.tensor_tensor(out=tmps[0][:, 0, :],
                                        in0=tmps[0][:, 0, :], in1=tmps[0][:, 1, :],
                                        op=ALU.add)
                nc.vector.tensor_tensor(out=tmps[1][:, 0, :],
                                        in0=tmps[1][:, 0, :], in1=tmps[1][:, 1, :],
                                        op=ALU.add)
                nc.vector.tensor_tensor(out=tmps[0][:, 0, :],
                                        in0=tmps[0][:, 0, :], in1=tmps[1][:, 0, :],
                                        op=ALU.add)
                ts4f = tmps[0][:, 0, :]
                # w tree: 8 -> 4 -> 2 -> 1
                r4 = redp.tile([128, H, 4, 32], F16, tag="r4")
                nc.vector.tensor_tensor(
                    out=r4[:, :, :, :].rearrange("p h w d -> p (h w d)"),
                    in0=bap(ts4f, [ts4f.ap[0], [W * 32, H], [32, 4], [1, 32]], off=ts4f.offset),
                    in1=bap(ts4f, [ts4f.ap[0], [W * 32, H], [32, 4], [1, 32]],
                            off=ts4f.offset + 4 * 32),
                    op=ALU.add)
                r4f = r4[:, :, :, :].rearrange("p h w d -> p (h w d)")
                r2 = redp.tile([128, H, 2, 32], F16, tag="r2")
                nc.vector.tensor_tensor(
                    out=r2[:, :, :, :].rearrange("p h w d -> p (h w d)"),
                    in0=bap(r4f, [r4f.ap[0], [4 * 32, H], [32, 2], [1, 32]], off=r4f.offset),
                    in1=bap(r4f, [r4f.ap[0], [4 * 32, H], [32, 2], [1, 32]],
                            off=r4f.offset + 2 * 32),
                    op=ALU.add)
                r2f = r2[:, :, :, :].rearrange("p h w d -> p (h w d)")
                rn = work.tile([128, H, 32], F16, tag="rn")
                nc.vector.tensor_tensor(
                    out=rn[:, :, :].rearrange("p h d -> p (h d)"),
                    in0=bap(r2f, [r2f.ap[0], [2 * 32, H], [1, 32]], off=r2f.offset),
                    in1=bap(r2f, [r2f.ap[0], [2 * 32, H], [1, 32]], off=r2f.offset + 32),
                    op=ALU.add)
                # normalize by softmax denominator
                att = work.tile([128, D], F32, tag="att")
                nc.vector.tensor_tensor(
                    out=att[:, :],
                    in0=rn[:, :, :].rearrange("p h d -> p (h d)"),
                    in1=bap(rd_ap, [rd_ap.ap[0], [1, H], [0, 32]], off=rd_ap.offset + ti * H),
                    op=ALU.mult)
                # out projection + residual (X += Wo @ att + bo)
                atb = work.tile([128, 2, 128], F16, tag="atb")
                for k in range(2):
                    trp = pt.tile([128, 128], F32, tag="pt")
                    nc.tensor.transpose(out=trp[:, :], in_=att[:, k * 128:(k + 1) * 128],
                                        identity=IDEN[:, :])
                    nc.scalar.activation(atb[:, k, :], trp[:, :], AF.Copy)
                for m in range(2):
                    ps = pt.tile([128, 128], F32, tag="pt")
                    for k in range(2):
                        nc.tensor.matmul(ps[:, :], lhsT=WOT[:, k, m * 128:(m + 1) * 128],
                                         rhs=atb[:, k, :], start=(k == 0), stop=False)
                    nc.tensor.matmul(ps[:, :], lhsT=BOR[:, m, :], rhs=ONESR[:, 0:128],
                                     start=False, stop=True)
                    nc.vector.tensor_tensor(out=X[:, m, s], in0=X[:, m, s],
                                            in1=ps[:, :], op=ALU.add)

            # ---- LN1 ----
            layer_norm(X, XH, [G1[:, 0, :], G1[:, 1, :]], [BE1[:, 0, :], BE1[:, 1, :]])

            # ---- FFN + residual into X ----
            for c in range(NTC):
                s = slice(c * TC, (c + 1) * TC)
                h1 = hpool.tile([128, FF // 128, TC], F16, tag="h1")
                for m in range(FF // 128):
                    ps = pffn.tile([128, TC], F32, tag="ffn")
                    for k in range(2):
                        nc.tensor.matmul(ps[:, :], lhsT=W1T[:, k, m * 128:(m + 1) * 128],
                                         rhs=XH[:, k, s], start=(k == 0), stop=(k == 1))
                    nc.scalar.activation(h1[:, m, :], ps[:, :], AF.Relu, bias=B1[:, m, :])
                for m in range(2):
                    ps = pffn.tile([128, TC], F32, tag="ffn")
                    for k in range(FF // 128):
                        nc.tensor.matmul(ps[:, :], lhsT=W2T[:, k, m * 128:(m + 1) * 128],
                                         rhs=h1[:, k, :], start=(k == 0), stop=False)
                    nc.tensor.matmul(ps[:, :], lhsT=B2R[:, m, :], rhs=ONESR[:, :],
                                     start=False, stop=True)
                    nc.vector.tensor_tensor(out=X[:, m, s], in0=XH[:, m, s],
                                            in1=ps[:, :], op=ALU.add)

            layer_norm(X, X, [G2[:, 0, :], G2[:, 1, :]], [BE2[:, 0, :], BE2[:, 1, :]])

        # ---- write result ----
        nc.sync.dma_start(out=out_d[:, :, :].rearrange("k p t -> p k t"), in_=X[:, :, :])

    ctx.close()
    nc.finalize()
    return nc


def _prep_core_inputs(b, srcs, poss, masks, level_embed, W_off, b_off, W_aw, b_aw,
                      W_val, b_val, W_out, b_out, g1, be1, W1, b1, W2, b2, g2, be2):
    f32 = np.float32
    x0 = np.concatenate([s[b] for s in srcs], axis=1).astype(f32)       # [256, T]
    pos = np.concatenate(
        [p[b] + level_embed[i][:, None] for i, p in enumerate(poss)], axis=1
    ).astype(f32)                                                        # [256, T]
    vr = np.stack([m[b].sum() / m.shape[1] for m in masks]).astype(f32)  # [NL]
    ref1 = np.concatenate(
        [(np.arange(Tl, dtype=f32) + 0.5) / Tl for Tl in LENS]
    )                                                                    # [T]
    # raw sampling base position per (q, l): ref*T_l - 0.5
    pbq = ref1[:, None] * vr[None, :] * np.array(LENS, f32)[None, :] - 0.5  # [T, NL]
    Sl = np.clip(np.rint(pbq) - (W // 2), 0, np.array(LENS)[None, :] - W)   # [T, NL]
    rows = (Sl + np.array(LB)[None, :]).astype(np.int64)                    # [T, NL]
    # dma_gather index layout: per (tile, level-pair): 256 idxs, idx i at
    # [i % 16, tile, pair, i // 16]; i < 128 -> level 2*pair, else 2*pair+1
    wsi = rows.astype(np.int32).reshape(QT, 128, NL).transpose(1, 0, 2).copy()

    def ktile(w):  # [din, dout] -> [2, 128, dout]
        return np.ascontiguousarray(w.reshape(2, 128, -1))

    wvt = np.stack([ktile(W_val[l].T) for l in range(LAYERS)])
    woa_w = [np.concatenate([W_off[l], W_aw[l]], axis=0) for l in range(LAYERS)]
    woa = np.stack([ktile(w.T) for w in woa_w])
    # poa[l, q, out]: off half gets pb - S baked in (window-local positions)
    pbl = (pbq - Sl).astype(f32)                                         # [T, NL]
    poa = np.stack([
        (woa_w[l] @ pos).T + np.concatenate([b_off[l], b_aw[l]])[None, :]
        for l in range(LAYERS)
    ])                                                                    # [L, T, 256]
    # slot s = h*16 + l*4 + p -> level index (s//4) % 4
    lidx = (np.arange(HLP) // NP) % NL
    poa[:, :, :HLP] += pbl[None, :, lidx]
    poa = poa.astype(np.float16).reshape(LAYERS, QT, 128, 2 * HLP)
    # kernel reads poa as [128 part(q within tile), QT*256]
    poa = np.ascontiguousarray(poa.transpose(0, 2, 1, 3).reshape(LAYERS, 128, QT * 2 * HLP))

    sel = np.zeros((8, NTC, 128), np.float16)
    for c in range(NTC):
        sel[c, c, :] = 1.0
    stsel = np.zeros((128, NTC, 8), np.float16)
    for c in range(NTC):
        stsel[:, c, c] = 1.0

    wot = np.stack([ktile(W_out[l].T) for l in range(LAYERS)])
    bo = b_out.reshape(LAYERS, 2, 128, 1).astype(f32)
    w1t = np.stack([ktile(W1[l].T) for l in range(LAYERS)]).astype(np.float16)
    b1r = b1.reshape(LAYERS, FF // 128, 128, 1).astype(f32)
    w2t = np.stack([np.ascontiguousarray(W2[l].T.reshape(FF // 128, 128, D))
                    for l in range(LAYERS)]).astype(np.float16)
    b2r = b2.reshape(LAYERS, 2, 128, 1).astype(f32)
    return {
        "x0": x0.reshape(2, 128, T).astype(np.float16),
        "poa": poa, "wsi": wsi, "sel": sel, "stsel": stsel,
        "wvt": wvt.astype(np.float16), "woa": woa.astype(np.float16),
        "wot": wot.astype(np.float16), "bo": bo,
        "bor": b_out.reshape(LAYERS, 1, 2, 128).astype(np.float16),
        "b2r": b2.reshape(LAYERS, 1, 2, 128).astype(np.float16),
        "w1t": w1t, "b1": b1r,
        "w2t": w2t, "b2": b2r,
        "g1": g1.reshape(LAYERS, 2, 128, 1).astype(f32),
        "be1": be1.reshape(LAYERS, 2, 128, 1).astype(f32),
        "g2": g2.reshape(LAYERS, 2, 128, 1).astype(f32),
        "be2": be2.reshape(LAYERS, 2, 128, 1).astype(f32),
    }


_NC_CACHE = {}


def _collect_args(inputs):
    return dict(
        srcs=[inputs[f"src{i}"] for i in range(4)],
        poss=[inputs[f"pos{i}"] for i in range(4)],
        masks=[inputs[f"mask{i}"] for i in range(4)],
        level_embed=inputs["level_embed"],
        W_off=inputs["W_off"], b_off=inputs["b_off"],
        W_aw=inputs["W_aw"], b_aw=inputs["b_aw"],
        W_val=inputs["W_val"], b_val=inputs["b_val"],
        W_out=inputs["W_out"], b_out=inputs["b_out"],
        g1=inputs["g1"], be1=inputs["be1"],
        W1=inputs["W1"], b1=inputs["b1"],
        W2=inputs["W2"], b2=inputs["b2"],
        g2=inputs["g2"], be2=inputs["be2"],
    )


def kernel(**inputs):
    inputs = {k: np.asarray(v) for k, v in inputs.items()}
    args = _collect_args(inputs)
    if "nc" not in _NC_CACHE:
        _NC_CACHE["nc"] = build_program()
    nc = _NC_CACHE["nc"]
    in_maps = [_prep_core_inputs(b, **args) for b in range(B)]
    for attempt in range(2):
        res = run_bass_kernel_spmd(nc, in_maps, core_ids=list(range(B)))
        outs = []
        for b in range(B):
            o = res.results[b]["out"]          # [2, 128, T] f16
            outs.append(o.reshape(D, T).T.astype(np.float32))
        out = np.stack(outs)
        if np.isfinite(out).all():
            return out
    return out


if __name__ == "__main__":
    np.random.seed(0)
    build_program()
    print("program built OK")
